# revision 1
# baseline (speedup 1.0000x reference)
"""Trainium2 Bass kernel for a ViT/Swin-style transformer block.

Strategy: pure data-parallel over batch (64 -> 8 per core), no collectives.
On-device layout is feature-major ("transposed"): activations live as
[features(partitions, k-tiles of 128), tokens(free)].  All GEMMs run in bf16
with fp32 PSUM accumulation.  LayerNorm affine params and attention scale /
gamma factors are folded into the weights on the host; host also pre-gathers
the relative-position-bias table into dense per-head [m, n] maps.

Perf structure:
  - QKV / V / proj / fc1 / fc2 GEMMs run fp8e4m3 in DoubleRow perf mode
    (2 contract k-tiles per instruction, 2x bf16 throughput).  Weights are
    scaled x64 on the host (keeps them out of fp8 subnormals); the 1/64
    descale folds into the PSUM-drain activations.  Residual stream,
    LayerNorm, scores, softmax and AV stay bf16/fp32.
  - Per-batch-element token blocks padded 197 -> 200 (even matmul free
    dims stream 2 cols/cycle; offsets stay 4B-aligned for dual-fp8
    ldweights).
  - rpb added to scores via identity matmuls accumulated into the score
    PSUM (PE) instead of DVE adds.
  - Softmax denominator rides the AV matmul as output row 64 (ones column
    in the v blocks).
  - Broadcasts (LN scale/shift rows, softmax 1/denom) run on GpSimd/Pool
    via partition_broadcast; LN stat squares run on the Scalar engine.
  - Weight DMAs issue from the Sync queue (waits there block no compute);
    w1/w2/wp prefetch overlaps attention.
  - LN chunks are software-pipelined (stats of chunk c+1 overlap
    apply/consume of chunk c); MLP runs 2 chunks behind LN2.
  - HW quirks found the hard way: dual-fp8 ldweights needs full-128-col
    stationaries and 4B-aligned offsets; pool-tag reuse with a dtype
    change races (dedicated tiles instead); DVE STT reading PSUM faults in
    some contexts (drain via Scalar ACT first); reciprocal_approx_fast
    needs base partition 0.

SBUF long-lived slots (tag reuse serialized by Tile dependency tracking;
reuse only ever with the SAME dtype -- dtype-changing reuse races):
  tg_x : xT bf16 residual input               (A..C)
  tg_1 : kT bf16 (A..B)  -> xT2 bf16 residual-1 output (C..D)
  tg_2 : qT bf16         (A..B)
  tg_3 : v blocks bf16   (A..B)
  tg_4 : aoT fp8 (B..C)  -> h2 fp8 (C..D)
  tg_5 : h1 fp8          (A)
w1 / w2 / wp / g are dedicated fp8 tiles (no slot reuse).
"""

import numpy as np
import ml_dtypes
from contextlib import ExitStack

import concourse.bacc as bacc
import concourse.bass as bass
import concourse.mybir as mybir
import concourse.tile as tile
from concourse.bass_utils import run_bass_kernel_spmd

bf16 = ml_dtypes.bfloat16
dt = mybir.dt
AF = mybir.ActivationFunctionType
ALU = mybir.AluOpType

# ---- problem dims (hardcoded) ----
B, N, D, H, DH, HID = 64, 197, 768, 12, 64, 3072
NCORES = 8
BPC = B // NCORES          # 8 batch elements per core
NP = 200                   # padded tokens per batch element (even free
                           # dims; multiples of 4 bytes for dual-fp8 ldweights)
T = BPC * NP               # 1600 token-columns per core
KT = D // 128              # 6 feature k-tiles
HT = HID // 128            # 24 hidden tiles
NCHUNK = 4
CHUNK = T // NCHUNK        # 400
MT = 2                     # m-tiles per batch element (128 + 69)
MSZ = [128, N - 128]       # [128, 69]
EPS = 1e-5

_NC_CACHE = {}


Q8 = P8 = M8 = True        # fp8 DoubleRow for QKV/V, proj, MLP


def _build_nc(has_pb=False, has_b2=False):
    key = (has_pb, has_b2)
    if key in _NC_CACHE:
        return _NC_CACHE[key]
    nc = bacc.Bacc(None, target_bir_lowering=False)
    FS = 1.0 / 64.0                     # fp8 weight descale

    # ---- DRAM I/O ----
    d_xT = nc.dram_tensor("xT", [D, T], dt.bfloat16, kind="ExternalInput")
    d_wqkv = nc.dram_tensor("wqkvT", [D, 3 * D], dt.float8e4 if Q8 else dt.bfloat16, kind="ExternalInput")
    d_wp = nc.dram_tensor("wpT", [D, D], dt.float8e4 if P8 else dt.bfloat16, kind="ExternalInput")
    d_w1 = nc.dram_tensor("w1T", [D, HID], dt.float8e4 if M8 else dt.bfloat16, kind="ExternalInput")
    d_w2 = nc.dram_tensor("w2T", [HID, D], dt.float8e4 if M8 else dt.bfloat16, kind="ExternalInput")
    d_qb = nc.dram_tensor("qb", [128, KT], dt.float32, kind="ExternalInput")
    d_kb = nc.dram_tensor("kb", [128, KT], dt.float32, kind="ExternalInput")
    d_vb = nc.dram_tensor("vb", [1, D], dt.bfloat16, kind="ExternalInput")
    d_pb = nc.dram_tensor("pb", [128, KT], dt.float32, kind="ExternalInput")
    d_b1 = nc.dram_tensor("b1", [128, HT], dt.float32, kind="ExternalInput")
    d_b2 = nc.dram_tensor("b2", [128, KT], dt.float32, kind="ExternalInput")
    d_id = nc.dram_tensor("ident", [128, 128], dt.bfloat16, kind="ExternalInput")
    d_rpb = nc.dram_tensor("rpbT", [128, H, MT * NP], dt.bfloat16, kind="ExternalInput")
    d_yT = nc.dram_tensor("yT", [D, T], dt.float32, kind="ExternalOutput")

    with ExitStack() as ctx:
        tc = ctx.enter_context(tile.TileContext(nc))

        p_const = tc.alloc_tile_pool(name="const", bufs=1)
        p_rows = tc.alloc_tile_pool(name="prows", bufs=2)
        p_big = tc.alloc_tile_pool(name="pbig", bufs=1)

        # constants
        ones_mu = p_const.tile([128, 1], dt.bfloat16)      # 1/768 for mean sums
        eps_t = p_const.tile([1, 1], dt.float32)
        nc.vector.memset(ones_mu[:], 1.0 / D)
        nc.vector.memset(eps_t[:], EPS)
        t_qb = p_const.tile([128, KT], dt.float32)
        t_kb = p_const.tile([128, KT], dt.float32)
        t_vb = p_const.tile([1, D], dt.bfloat16)
        t_pb = p_const.tile([128, KT], dt.float32)
        t_b1 = p_const.tile([128, HT], dt.float32)
        t_b2 = p_const.tile([128, KT], dt.float32)
        t_id = p_const.tile([128, 128], dt.bfloat16)
        fs_col = p_const.tile([128, 1], dt.float32)
        nc.vector.memset(fs_col[:], FS)

        # long-lived slots
        xT = p_big.tile([128, KT, T], dt.bfloat16, tag="tg_x")
        xTr = d_xT.rearrange("(k p) t -> p k t", p=128)
        kTt = p_big.tile([128, KT, T], dt.bfloat16, tag="tg_1")
        qT = p_big.tile([128, KT, T], dt.bfloat16, tag="tg_2")
        # v token-major per-head blocks of 65 cols: cols 0..63 = v,
        # col 64 = ones (softmax denominator rides AV matmul as out row 64).
        vtok = p_big.tile([128, BPC, MT, H, 65], dt.bfloat16, tag="tg_3")
        for h in range(H):
            nc.vector.memset(vtok[:, :, :, h, 64:65], 1.0)
        h1 = p_big.tile([128, KT, T], dt.float8e4 if Q8 else dt.bfloat16, tag="tg_5")
        rpb = p_big.tile([128, H, MT * NP], dt.bfloat16, tag="tg_rpb")

        # sync-queue DMA order: x c0, consts, wqkv-qk, x c1..c3, wqkv-v,
        # rpb, wp, (later) w1, w2, y stores.
        nc.sync.dma_start(xT[:, :, bass.ts(0, CHUNK)], xTr[:, :, bass.ts(0, CHUNK)])
        for t_, d_ in [(t_qb, d_qb), (t_kb, d_kb), (t_vb, d_vb), (t_pb, d_pb),
                       (t_b1, d_b1), (t_b2, d_b2), (t_id, d_id)]:
            nc.sync.dma_start(t_[:], d_[:])
        p_qkvw = tc.alloc_tile_pool(name="pqkvw", bufs=1)
        wqkv = p_qkvw.tile([128, KT, 3 * D], dt.float8e4 if Q8 else dt.bfloat16)
        wqkvr = d_wqkv.rearrange("(k p) m -> p k m", p=128)
        nc.sync.dma_start(wqkv[:, :, 0:2 * D], wqkvr[:, :, 0:2 * D])
        for c in range(1, NCHUNK):
            cs = bass.ts(c, CHUNK)
            nc.sync.dma_start(xT[:, :, cs], xTr[:, :, cs])
        nc.sync.dma_start(wqkv[:, :, 2 * D:3 * D], wqkvr[:, :, 2 * D:3 * D])
        nc.sync.dma_start(rpb[:], d_rpb[:])

        # v-bias broadcast to all partitions (feature-varying row)
        vb_full = p_const.tile([128, D], dt.bfloat16)
        nc.gpsimd.partition_broadcast(vb_full[:], t_vb[:])

        # ============ LayerNorm helper pieces (feature-major) ============
        def ln_copies(tmp_pool, src_bf, c):
            cs = bass.ts(c, CHUNK)
            x2 = tmp_pool.tile([128, KT, CHUNK], dt.bfloat16, tag="x2", bufs=1)
            for k in range(KT):
                nc.scalar.square(x2[:, k, :], src_bf[:, k, cs])
            return x2

        def ln_stats(psum_pool, src_bf, c, x2):
            cs = bass.ts(c, CHUNK)
            mu_ps = psum_pool.tile([1, CHUNK], dt.float32, tag="stat", bufs=4)
            ms_ps = psum_pool.tile([1, CHUNK], dt.float32, tag="stat", bufs=4)
            for k in range(KT):
                nc.tensor.matmul(mu_ps[:], ones_mu[:], src_bf[:, k, cs],
                                 start=(k == 0), stop=(k == KT - 1))
                nc.tensor.matmul(ms_ps[:], ones_mu[:], x2[:, k, :],
                                 start=(k == 0), stop=(k == KT - 1))
            musq = p_rows.tile([1, CHUNK], dt.float32, tag="musq")
            nc.scalar.square(musq[:], mu_ps[:])
            var = p_rows.tile([1, CHUNK], dt.float32, tag="var")
            nc.vector.tensor_sub(var[:], ms_ps[:], musq[:])
            std = p_rows.tile([1, CHUNK], dt.float32, tag="std")
            nc.scalar.activation(std[:], var[:], AF.Sqrt, bias=eps_t[0:1, 0:1])
            a_f = p_rows.tile([1, CHUNK], dt.float32, tag="af")
            nc.vector.reciprocal_approx_fast(a_f[:], std[:])
            b_f = p_rows.tile([1, CHUNK], dt.float32, tag="bf")
            nc.vector.scalar_tensor_tensor(b_f[:], mu_ps[:], -1.0, a_f[:],
                                           op0=ALU.mult, op1=ALU.mult)
            return a_f, b_f

        def ln_apply(tmp_pool, src_bf, dst_bf, c, a_f, b_f):
            cs = bass.ts(c, CHUNK)
            bc_a = tmp_pool.tile([128, CHUNK], dt.bfloat16, tag="bca", bufs=2)
            bc_b = tmp_pool.tile([128, CHUNK], dt.bfloat16, tag="bcb", bufs=2)
            a_b = p_rows.tile([1, CHUNK], dt.bfloat16, tag="afb")
            b_b = p_rows.tile([1, CHUNK], dt.bfloat16, tag="bfb")
            with nc.allow_low_precision(reason="ln rows bf16"):
                nc.vector.tensor_copy(a_b[:], a_f[:])
                nc.vector.tensor_copy(b_b[:], b_f[:])
            nc.gpsimd.partition_broadcast(bc_a[:], a_b[:])
            nc.gpsimd.partition_broadcast(bc_b[:], b_b[:])
            for k in range(KT):
                tmp = tmp_pool.tile([128, CHUNK], dt.float32, tag="ntmp", bufs=2)
                nc.vector.tensor_mul(tmp[:], src_bf[:, k, cs], bc_a[:])
                with nc.allow_low_precision(reason="ln out fp8"):
                    nc.vector.tensor_add(dst_bf[:, k, cs], tmp[:], bc_b[:])


        def mm_chain(ps_t, lhs_fn, rhs_fn, dual):
            """lhs_fn/rhs_fn: k -> AP (single k-tile); dual pairs them."""
            if dual:
                for kp in range(KT // 2):
                    nc.tensor.matmul(ps_t, lhs_fn(2 * kp, 2), rhs_fn(2 * kp, 2),
                                     start=(kp == 0), stop=(kp == KT // 2 - 1),
                                     perf_mode=mybir.MatmulPerfMode.DoubleRow)
            else:
                for k in range(KT):
                    nc.tensor.matmul(ps_t, lhs_fn(k, 1), rhs_fn(k, 1),
                                     start=(k == 0), stop=(k == KT - 1))

        # ============ Phase A: LN1 + QKV (1-chunk software pipeline) ========
        p_atmp = tc.alloc_tile_pool(name="patmp", bufs=1)
        psA = tc.alloc_tile_pool(name="psA", bufs=1, space="PSUM")

        def qkv_chunk(c):
            cs = bass.ts(c, CHUNK)
            for d_i in range(KT):
                pq = psA.tile([128, CHUNK], dt.float32, tag="mm", bufs=4)
                mm_chain(pq[:],
                         lambda k, n: wqkv[:, k:k + n, bass.ts(d_i, 128)],
                         lambda k, n: h1[:, k:k + n, cs], Q8)
                nc.scalar.activation(qT[:, d_i, cs], pq[:], AF.Identity,
                                     bias=t_qb[:, d_i:d_i + 1], scale=FS)
            for d_i in range(KT):
                pk = psA.tile([128, CHUNK], dt.float32, tag="mm", bufs=4)
                mm_chain(pk[:],
                         lambda k, n: wqkv[:, k:k + n,
                                           D + d_i * 128:D + d_i * 128 + 128],
                         lambda k, n: h1[:, k:k + n, cs], Q8)
                nc.scalar.activation(kTt[:, d_i, cs], pk[:], AF.Identity,
                                     bias=t_kb[:, d_i:d_i + 1], scale=FS)

        def v_chunk(c):
            for b in (2 * c, 2 * c + 1):
                for mt in range(MT):
                    msz = MSZ[mt]
                    n0 = b * NP + mt * 128
                    for half in range(2):
                        pv = psA.tile([128, 384], dt.float32, tag="mm", bufs=4)
                        mm_chain(pv[0:msz, :],
                                 lambda k, n: h1[:, k:k + n, n0:n0 + msz],
                                 lambda k, n: wqkv[:, k:k + n,
                                                   2 * D + half * 384:2 * D + half * 384 + 384],
                                 Q8 and msz == 128)
                        with nc.allow_low_precision(reason="v bf16 store"):
                            nc.vector.scalar_tensor_tensor(
                                vtok[0:msz, b, mt, half * 6:(half + 1) * 6, 0:64],
                                pv[0:msz, :].rearrange("p (h w) -> p h w", w=64),
                                FS,
                                vb_full[0:msz, half * 384:(half + 1) * 384].rearrange(
                                    "p (h w) -> p h w", w=64),
                                op0=ALU.mult, op1=ALU.add)

        lnA = {}
        for c in range(NCHUNK + 1):
            if c >= 1:
                a_f, b_f = lnA.pop(c - 1)
                ln_apply(p_atmp, xT, h1, c - 1, a_f, b_f)
                qkv_chunk(c - 1)
            if c < NCHUNK:
                x2 = ln_copies(p_atmp, xT, c)
                a_f, b_f = ln_stats(psA, xT, c, x2)
                lnA[c] = (a_f, b_f)
            if c >= 1:
                v_chunk(c - 1)

        psA.release()
        p_atmp.release()
        p_qkvw.release()

        # proj + MLP w1 prefetch: issued post-A on the sync queue (wp
        # first -- needed at the first interleaved proj chunk).
        p_dw = tc.alloc_tile_pool(name="pdw", bufs=1)
        p_wp = tc.alloc_tile_pool(name="pwp", bufs=1)
        wp = p_wp.tile([128, KT, D], dt.float8e4 if P8 else dt.bfloat16)
        nc.sync.dma_start(wp[:], d_wp.rearrange("(k p) m -> p k m", p=128))
        w1 = p_dw.tile([128, KT, HID], dt.float8e4 if M8 else dt.bfloat16)
        nc.sync.dma_start(w1[:], d_w1.rearrange("(k p) m -> p k m", p=128))
        w2 = p_dw.tile([128, HT, D], dt.float8e4 if M8 else dt.bfloat16)
        nc.sync.dma_start(w2[:], d_w2.rearrange("(k p) m -> p k m", p=128))

        # ============ Phase B: attention + interleaved proj =================
        aoT = p_big.tile([128, KT, T], dt.float8e4 if P8 else dt.bfloat16, tag="tg_4")
        p_aw = tc.alloc_tile_pool(name="paw", bufs=2)
        psB = tc.alloc_tile_pool(name="psB", bufs=1, space="PSUM")
        psP = tc.alloc_tile_pool(name="psP", bufs=1, space="PSUM")

        def vaug_ap(b, mt, h):
            """lhsT [msz, 65]: 64 v columns + ones column.
            AV output rows 0..63 = head AV, row 64 = softmax denominator."""
            return vtok[0:MSZ[mt], b, mt, h, :]

        PAIRS = [(b, j) for b in range(BPC) for j in range(H // 2)]
        state = {}

        def stage0(p):                       # scores + rpb -> psum (PE)
            b, j = p
            ts_n = slice(b * NP, (b + 1) * NP)
            scs = {}
            for e in range(2):
                hp = e * 64
                sc = psB.tile([128, MT * NP], dt.float32, tag="sc", bufs=4,
                              name=f"sc_{b}_{j}_{e}")
                for mt in range(MT):
                    msz = MSZ[mt]
                    m0 = b * NP + mt * 128
                    nc.tensor.matmul(sc[0:msz, bass.ts(mt, NP)],
                                     kTt[hp:hp + 64, j, m0:m0 + msz],
                                     qT[hp:hp + 64, j, ts_n],
                                     start=True, stop=False)
                    nc.tensor.matmul(sc[0:msz, bass.ts(mt, NP)],
                                     t_id[:, 0:msz],
                                     rpb[:, 2 * j + e, bass.ts(mt, NP)],
                                     start=False, stop=True)
                scs[e] = sc
            state[p] = {"scs": scs}

        def stage1(p):                       # exp -> p_bf (ACT)
            b, j = p
            st = state[p]
            p_bf = p_aw.tile([128, 2, MT * NP], dt.bfloat16, tag="pbf",
                             name=f"pbf_{b}_{j}")
            for e in range(2):
                for mt in range(MT):
                    msz = MSZ[mt]
                    nc.scalar.activation(p_bf[0:msz, e, bass.ts(mt, NP)],
                                         st["scs"][e][0:msz, bass.ts(mt, NP)],
                                         AF.Exp)
            st["p_bf"] = p_bf

        def stage2(p):                       # AV + denom (PE)
            b, j = p
            st = state[p]
            av = psB.tile([128, 2 * NP], dt.float32, tag="av", bufs=2,
                          name=f"av_{b}_{j}")
            for e in range(2):
                h = 2 * j + e
                for mt in range(MT):
                    nc.tensor.matmul(av[0:65, e * NP:e * NP + NP], vaug_ap(b, mt, h),
                                     st["p_bf"][0:MSZ[mt], e, mt * NP:mt * NP + NP],
                                     start=(mt == 0), stop=(mt == MT - 1))
            st["av"] = av

        def stage3(p):                       # recip, pool-bcast, normalize
            b, j = p
            ts_n = slice(b * NP, (b + 1) * NP)
            st = state[p]
            av = st["av"]
            rr = p_aw.tile([1, 2 * NP], dt.float32, tag="rr", name=f"rr_{b}_{j}")
            nc.vector.tensor_copy(rr[:], av[64:65, :])
            rc = p_aw.tile([1, 2 * NP], dt.float32, tag="rc", name=f"rc_{b}_{j}")
            nc.vector.reciprocal_approx_fast(rc[:], rr[:])
            rcb = p_aw.tile([1, 2 * NP], dt.bfloat16, tag="rcb", name=f"rcb_{b}_{j}")
            with nc.allow_low_precision(reason="softmax 1/denom bf16"):
                nc.scalar.copy(rcb[:], rc[:])
            bcs = p_aw.tile([64, 2 * NP], dt.bfloat16, tag="bcs",
                            name=f"bcs_{b}_{j}")
            nc.gpsimd.partition_broadcast(bcs[:], rcb[:])
            with nc.allow_low_precision(reason="attn out fp8"):
                nc.vector.tensor_mul(aoT[0:64, j, ts_n],
                                     av[0:64, 0:NP], bcs[0:64, 0:NP])
                nc.vector.tensor_mul(aoT[64:128, j, ts_n],
                                     av[0:64, NP:2 * NP], bcs[0:64, NP:2 * NP])
            del state[p]

        def proj_chunk(c):
            cs = bass.ts(c, CHUNK)
            for d_i in range(KT):
                pp = psP.tile([128, CHUNK], dt.float32, tag="pm", bufs=2)
                mm_chain(pp[:],
                         lambda k, n: wp[:, k:k + n, bass.ts(d_i, 128)],
                         lambda k, n: aoT[:, k:k + n, cs], P8)
                ptmp = p_aw.tile([128, CHUNK], dt.bfloat16, tag="ptmp",
                                 bufs=2, name=f"ptmp_{c}_{d_i}")
                with nc.allow_low_precision(reason="residual bf16"):
                    nc.scalar.activation(ptmp[:], pp[:], AF.Identity, scale=FS)
                    nc.vector.scalar_tensor_tensor(
                        xT2[:, d_i, cs], ptmp[:], t_pb[:, d_i:d_i + 1],
                        xT[:, d_i, cs], op0=ALU.add, op1=ALU.add)

        NPAIR = len(PAIRS)
        for i in range(NPAIR + 2):
            if i - 2 >= 0:
                stage3(PAIRS[i - 2])
            if 0 <= i - 1 < NPAIR:
                stage2(PAIRS[i - 1])
            if i < NPAIR:
                stage0(PAIRS[i])
                stage1(PAIRS[i])

        # residual-1 output goes to a fresh buffer in the freed kTt slot
        # (the in-place mult-STT form faults the DVE).
        xT2 = p_big.tile([128, KT, T], dt.bfloat16, tag="tg_1")
        for c in range(NCHUNK):
            proj_chunk(c)

        psP.release()
        psB.release()
        p_aw.release()
        p_wp.release()

        # w2 prefetch into the qT slot (free after the last stage0)
        w2 = p_big.tile([128, HT, D], dt.float8e4 if M8 else dt.bfloat16, tag="tg_2")
        nc.sync.dma_start(w2[:], d_w2.rearrange("(k p) m -> p k m", p=128))

        # ============ Phase C+D: LN2 (2-chunk lookahead) + MLP ==============
        h2 = p_big.tile([128, KT, T], dt.float8e4 if M8 else dt.bfloat16, tag="tg_4")
        p_ctmp = tc.alloc_tile_pool(name="pctmp", bufs=1)
        p_g = tc.alloc_tile_pool(name="pg", bufs=1)
        p_y = tc.alloc_tile_pool(name="py", bufs=2)
        psC = tc.alloc_tile_pool(name="psC", bufs=1, space="PSUM")

        def mlp_pair(P):
            c0, c1 = 2 * P, 2 * P + 1
            cps = [bass.ts(c0, CHUNK), bass.ts(c1, CHUNK)]
            g = p_g.tile([128, 2, HT, CHUNK],
                         dt.float8e4 if M8 else dt.bfloat16, tag="g")
            for hh in range(HT):
                pfs = [psD.tile([128, CHUNK], dt.float32, tag="f1", bufs=4,
                                name=f"pf_{P}_{hh}_{i}") for i in range(2)]
                for kp in range(KT // 2):
                    for i in range(2):
                        nc.tensor.matmul(
                            pfs[i][:], w1[:, 2 * kp:2 * kp + 2, bass.ts(hh, 128)],
                            h2[:, 2 * kp:2 * kp + 2, cps[i]],
                            start=(kp == 0), stop=(kp == KT // 2 - 1),
                            perf_mode=mybir.MatmulPerfMode.DoubleRow)
                for i in range(2):
                    with nc.allow_low_precision(reason="gelu fp8"):
                        nc.scalar.activation(g[:, i, hh, :], pfs[i][:], AF.Gelu,
                                             bias=t_b1[:, hh:hh + 1], scale=FS)
            for d_i in range(KT):
                pos = [psD.tile([128, CHUNK], dt.float32, tag="f2", bufs=4,
                                name=f"po_{P}_{d_i}_{i}") for i in range(2)]
                for hp in range(HT // 2):
                    for i in range(2):
                        nc.tensor.matmul(
                            pos[i][:], w2[:, 2 * hp:2 * hp + 2, bass.ts(d_i, 128)],
                            g[:, i, 2 * hp:2 * hp + 2, :],
                            start=(hp == 0), stop=(hp == HT // 2 - 1),
                            perf_mode=mybir.MatmulPerfMode.DoubleRow)
                for i in range(2):
                    ytmp = p_y.tile([128, CHUNK], dt.bfloat16, tag="yt", bufs=3)
                    with nc.allow_low_precision(reason="fc2 drain bf16"):
                        nc.scalar.activation(ytmp[:], pos[i][:], AF.Identity,
                                             scale=FS)
                    y = p_y.tile([128, CHUNK], dt.float32, tag="y", bufs=3)
                    nc.vector.scalar_tensor_tensor(y[:], ytmp[:],
                                                   t_b2[:, d_i:d_i + 1],
                                                   xT2[:, d_i, cps[i]],
                                                   op0=ALU.add, op1=ALU.add)
                    nc.sync.dma_start(
                        d_yT.rearrange("(k p) t -> p k t", p=128)[:, d_i, cps[i]],
                        y[:])

        lnS = []
        for c in range(NCHUNK):
            x2 = ln_copies(p_ctmp, xT2, c)
            lnS.append(ln_stats(psC, xT2, c, x2))
        for c in range(NCHUNK):
            a_f, b_f = lnS[c]
            ln_apply(p_ctmp, xT2, h2, c, a_f, b_f)
        psC.release()
        psD = tc.alloc_tile_pool(name="psD", bufs=1, space="PSUM")
        for P in range(NCHUNK // 2):
            mlp_pair(P)

        psD.release()
        p_y.release()
        p_g.release()
        p_ctmp.release()
        p_dw.release()
        p_big.release()
        p_rows.release()
        p_const.release()

    nc.finalize()
    _NC_CACHE[key] = nc
    return nc


def _prep_host(inputs):
    """Fold LN affines / scales / gammas into weights; build per-core in_maps."""
    f = np.float32
    x = np.asarray(inputs["x"], f)
    n1w, n1b = np.asarray(inputs["norm1_w"], f), np.asarray(inputs["norm1_b"], f)
    n2w, n2b = np.asarray(inputs["norm2_w"], f), np.asarray(inputs["norm2_b"], f)
    qkv_w = np.asarray(inputs["qkv_w"], f)
    q_bias, v_bias = np.asarray(inputs["q_bias"], f), np.asarray(inputs["v_bias"], f)
    rpb_table = np.asarray(inputs["rpb_table"], f)
    rel_index = np.asarray(inputs["rel_index"])
    proj_w, proj_b = np.asarray(inputs["proj_w"], f), np.asarray(inputs["proj_b"], f)
    g1, g2 = np.asarray(inputs["gamma1"], f), np.asarray(inputs["gamma2"], f)
    fc1_w, fc1_b = np.asarray(inputs["fc1_w"], f), np.asarray(inputs["fc1_b"], f)
    fc2_w, fc2_b = np.asarray(inputs["fc2_w"], f), np.asarray(inputs["fc2_b"], f)

    scale = DH ** -0.5
    f8q = ml_dtypes.float8_e4m3 if Q8 else bf16
    f8p = ml_dtypes.float8_e4m3 if P8 else bf16
    f8m = ml_dtypes.float8_e4m3 if M8 else bf16
    WS = 64.0
    Wq, Wk, Wv = qkv_w[0:D], qkv_w[D:2 * D], qkv_w[2 * D:3 * D]
    WqT = (WS * scale * (Wq * n1w[None, :]).T).astype(f8q)
    WkT = (WS * (Wk * n1w[None, :]).T).astype(f8q)
    WvT = (WS * (Wv * n1w[None, :]).T).astype(f8q)
    wqkvT = np.ascontiguousarray(np.concatenate([WqT, WkT, WvT], axis=1))
    qb = (scale * (Wq @ n1b + q_bias)).reshape(KT, 128).T.copy()   # [128, KT]
    kb = (Wk @ n1b).reshape(KT, 128).T.copy()
    vb = (Wv @ n1b + v_bias).reshape(1, D).astype(bf16)
    wpT = np.ascontiguousarray((WS * g1[:, None] * proj_w).T.astype(f8p))
    pb = (g1 * proj_b).reshape(KT, 128).T.copy()
    w1T = np.ascontiguousarray((WS * fc1_w * n2w[None, :]).T.astype(f8m))
    b1 = (fc1_w @ n2b + fc1_b).reshape(HT, 128).T.copy()
    w2T = np.ascontiguousarray((WS * g2[:, None] * fc2_w).T.astype(f8m))
    b2 = (g2 * fc2_b).reshape(KT, 128).T.copy()

    # rpbT[p, h, mt*NP+n] = rpb[h, n, m=mt*128+p]  (scoresT orientation)
    RPB = rpb_table[rel_index]            # [n, m, H]
    rpbT = np.zeros((128, H, MT * NP), f)
    for mt in range(MT):
        msz = MSZ[mt]
        blk = RPB[:, mt * 128:mt * 128 + msz, :].transpose(1, 2, 0)  # [m_sl, H, n]
        for h in range(H):
            rpbT[0:msz, h, mt * NP:mt * NP + N] = blk[:, h, :]
    rpbT = rpbT.astype(bf16)

    shared = dict(wqkvT=wqkvT, wpT=wpT, w1T=w1T, w2T=w2T,
                  qb=np.ascontiguousarray(qb), kb=np.ascontiguousarray(kb),
                  vb=vb, pb=np.ascontiguousarray(pb),
                  b1=np.ascontiguousarray(b1), b2=np.ascontiguousarray(b2),
                  ident=np.eye(128, dtype=bf16), rpbT=rpbT)
    in_maps = []
    for core in range(NCORES):
        xs = x[core * BPC:(core + 1) * BPC]            # [BPC, N, D]
        xp = np.zeros((BPC, NP, D), f)
        xp[:, 0:N, :] = xs
        xT = np.ascontiguousarray(xp.reshape(T, D).T).astype(bf16)  # [D, T]
        m = dict(shared)
        m["xT"] = xT
        in_maps.append(m)
    return in_maps


def kernel(**inputs) -> np.ndarray:
    in_maps = _prep_host(inputs)
    has_pb = bool(np.any(in_maps[0]["pb"]))
    has_b2 = bool(np.any(in_maps[0]["b2"]))
    nc = _build_nc(has_pb, has_b2)
    res = run_bass_kernel_spmd(nc, in_maps, core_ids=list(range(NCORES)))
    outs = []
    for core in range(NCORES):
        yT = res.results[core]["yT"]                   # [D, T]
        yp = np.asarray(yT, np.float32).T.reshape(BPC, NP, D)
        outs.append(yp[:, 0:N, :])
    return np.concatenate(outs, axis=0)

